# revision 4
# baseline (speedup 1.0000x reference)
"""Trainium2 Bass kernel for DifferentiableMVOLayer (batched simplex-constrained QP).

Per-sample FISTA solve of  min -mu'w + (lam/2) w'(U^T U)w  s.t. w in simplex.
Data-parallel over 8 NeuronCores (16 samples each).

Key design points:
  - Q = U^T U formed on-chip (fp32r matmuls, fp32 PSUM accumulate), kept in SBUF.
  - Batched matvec Q@y via masked-stationary trick: stationary [128,8] holds one
    sample's y slice in column b (zeros elsewhere) so 32 matmuls + an identity
    matmul folding -mu/lam accumulate all 8 samples' results into one PSUM tile
    [8, 512] in natural layout.
  - Simplex projection via warm-started Newton/Michelot threshold iteration
    (exact after a few steps), all on the vector engine with fused
    scalar_tensor_tensor/accum ops.
  - FISTA momentum scalars are input-independent -> baked in as immediates.
  - Two 8-sample groups pipeline PE (matvec) against DVE (projection).
"""

import math
import numpy as np

N_ASSETS = 512
BATCH = 128
N_CORES = 8
B_CORE = BATCH // N_CORES          # 16 samples per core
GRP = 8                            # samples per pipeline group
N_GROUPS = B_CORE // GRP
LAMBDA = 10.0
FISTA_ITERS = 300
POWER_ITERS = 16
MICH_COLD = 8                      # Michelot iters, first FISTA step
MICH_WARM = 3                      # Michelot iters, warm-started steps
NT = N_ASSETS // 128               # 4 j-tiles

_CACHE = {}


def _momentum_coeffs(n_iters):
    t = np.float32(1.0)
    cs = []
    for _ in range(n_iters):
        t_new = np.float32(0.5 * (1.0 + np.sqrt(np.float32(1.0 + 4.0 * t * t))))
        cs.append(float((t - np.float32(1.0)) / t_new))
        t = t_new
    return cs


def _build(n_fista, n_power, debug=False):
    import concourse.bass as bass
    import concourse.mybir as mybir
    import concourse.tile as tile
    import concourse.bacc as bacc

    F32 = mybir.dt.float32
    F32R = mybir.dt.float32r
    OP = mybir.AluOpType

    nc = bacc.Bacc(trn_type="TRN2", target_bir_lowering=False)
    mu_d = nc.dram_tensor("mu", [B_CORE, N_ASSETS], F32, kind="ExternalInput")
    u_d = nc.dram_tensor("U", [B_CORE, N_ASSETS, N_ASSETS], F32, kind="ExternalInput")
    w_d = nc.dram_tensor("W", [B_CORE, N_ASSETS], F32, kind="ExternalOutput")
    if debug:
        dbg_d = nc.dram_tensor("DBG", [B_CORE, N_ASSETS], F32, kind="ExternalOutput")
        dbg2_d = nc.dram_tensor("DBG2", [B_CORE, 8], F32, kind="ExternalOutput")

    inv_sqrt_n = 1.0 / math.sqrt(N_ASSETS)
    cs = _momentum_coeffs(n_fista)

    with tile.TileContext(nc) as tc:
        with (
            tc.tile_pool(name="big", bufs=1) as big,
            tc.tile_pool(name="small", bufs=1) as small,
            tc.tile_pool(name="ps", bufs=1, space="PSUM") as ps,
        ):
            # ---------------- static tiles ----------------
            qall = big.tile([128, B_CORE, NT, N_ASSETS], F32R, name="qall")
            mu_sb = small.tile([B_CORE, N_ASSETS], F32, name="mu_sb")
            negmulam = small.tile([B_CORE, N_ASSETS], F32R, name="negmulam")
            zeros8 = small.tile([GRP, N_ASSETS], F32, name="zeros8")
            i16tmp = small.tile([16, 16], F32, name="i16tmp")
            i16f = small.tile([16, 16], F32, name="i16f")
            i16r = small.tile([16, 16], F32R, name="i16r")
            vinit = small.tile([128, NT, GRP], F32, name="vinit")

            nc.sync.dma_start(mu_sb[:], mu_d[:])
            nc.vector.memset(zeros8[:], 0.0)
            nc.gpsimd.iota(i16tmp[:], pattern=[[1, 16]], base=0,
                           channel_multiplier=-1,
                           allow_small_or_imprecise_dtypes=True)
            nc.vector.tensor_scalar(i16f[:], i16tmp[:], 0.0, None, OP.is_equal)
            nc.vector.tensor_copy(i16r[:], i16f[:])
            nc.vector.memset(vinit[:], inv_sqrt_n)
            nc.vector.tensor_scalar(negmulam[:], mu_sb[:], -1.0 / LAMBDA, None,
                                    OP.mult)
            i8 = i16f[0:GRP, 0:GRP]

            # per-group state
            ymask, yv, wv, wprev, vv, trash, yT = [], [], [], [], [], [], []
            th, rr, cc, rc, dth, nega, pv = [], [], [], [], [], [], []
            for g in range(N_GROUPS):
                ymask.append(big.tile([128, NT, GRP, GRP], F32R, name=f"ymask{g}"))
                yv.append(small.tile([GRP, N_ASSETS], F32, name=f"y{g}"))
                wv.append(small.tile([GRP, N_ASSETS], F32, name=f"w{g}"))
                wprev.append(small.tile([GRP, N_ASSETS], F32, name=f"wprev{g}"))
                vv.append(small.tile([GRP, N_ASSETS], F32, name=f"v{g}"))
                trash.append(small.tile([GRP, N_ASSETS], F32, name=f"trash{g}"))
                th.append(small.tile([GRP, 1], F32, name=f"th{g}"))
                rr.append(small.tile([GRP, 1], F32, name=f"r{g}"))
                cc.append(small.tile([GRP, 1], F32, name=f"c{g}"))
                rc.append(small.tile([GRP, 1], F32, name=f"rc{g}"))
                dth.append(small.tile([GRP, 1], F32, name=f"dth{g}"))
                nega.append(small.tile([GRP, 1], F32, name=f"nega{g}"))
                pv.append(ps.tile([GRP, N_ASSETS], F32, name=f"pv{g}"))
                yT.append(ps.tile([128, NT, GRP], F32, name=f"yT{g}"))

            def ym_diag(g):
                return ymask[g][:].rearrange("p t a b -> p t (a b)")[:, :, 0:GRP * GRP:GRP + 1]

            # ---------------- phase A: Q = U^T U ----------------
            with (
                tc.tile_pool(name="stage", bufs=2) as stage_pool,
                tc.tile_pool(name="qps", bufs=4, space="PSUM") as qps_pool,
            ):
                for s in range(B_CORE):
                    ustage = stage_pool.tile([128, NT, N_ASSETS], F32,
                                             name="ustage", tag="ustage")
                    ur = stage_pool.tile([128, NT, N_ASSETS], F32R,
                                         name="ur", tag="ur")
                    nc.sync.dma_start(
                        ustage[:], u_d[s].rearrange("(t p) j -> p t j", p=128))
                    nc.vector.tensor_copy(ur[:], ustage[:])
                    for jm in range(NT):
                        qp = qps_pool.tile([128, N_ASSETS], F32, name="qp", tag="qp")
                        for it in range(NT):
                            nc.tensor.matmul(
                                qp[:], ur[:, it, jm * 128:(jm + 1) * 128],
                                ur[:, it, :],
                                start=(it == 0), stop=(it == NT - 1))
                        nc.vector.tensor_copy(qall[:, s, jm, :], qp[:])

            # ---------------- matvec helper ----------------
            def matvec(g, with_mu):
                for jt in range(NT):
                    for b in range(GRP):
                        s = g * GRP + b
                        last = (jt == NT - 1 and b == GRP - 1 and not with_mu)
                        nc.tensor.matmul(
                            pv[g][:], ymask[g][:, jt, :, b], qall[:, s, jt, :],
                            start=(jt == 0 and b == 0), stop=last)
                if with_mu:
                    nc.tensor.matmul(
                        pv[g][:], i16r[:, g * GRP:(g + 1) * GRP], negmulam[:],
                        start=False, stop=True)

            def retranspose(g, src):
                # src [GRP, 512] fp32 -> ymask diag (fp32r)
                for jt in range(NT):
                    nc.tensor.transpose(
                        yT[g][:, jt, :], src[:, jt * 128:(jt + 1) * 128], i8)
                nc.vector.tensor_copy(ym_diag(g), yT[g][:])

            # ---------------- phase B: power iteration ----------------
            qv = [small.tile([GRP, N_ASSETS], F32, name=f"qv{g}")
                  for g in range(N_GROUPS)]
            ss = [small.tile([GRP, 1], F32, name=f"ss{g}") for g in range(N_GROUPS)]
            sqs = [small.tile([GRP, 1], F32, name=f"sq{g}") for g in range(N_GROUPS)]

            for g in range(N_GROUPS):
                nc.vector.memset(ymask[g][:].bitcast(F32), 0.0)
                nc.vector.tensor_copy(ym_diag(g), vinit[:])

            for it in range(n_power):
                for g in range(N_GROUPS):
                    matvec(g, with_mu=False)
                for g in range(N_GROUPS):
                    nc.vector.tensor_copy(qv[g][:], pv[g][:])
                    nc.vector.scalar_tensor_tensor(
                        trash[g][:], qv[g][:], 0.0, qv[g][:], OP.add, OP.mult,
                        accum_out=ss[g][:])
                    nc.scalar.sqrt(sqs[g][:], ss[g][:])
                    nc.vector.tensor_scalar(sqs[g][:], sqs[g][:], 1e-12, None,
                                            OP.add)
                    nc.vector.reciprocal(rc[g][:], sqs[g][:])
                    nc.vector.tensor_scalar(qv[g][:], qv[g][:], rc[g][:], None,
                                            OP.mult)
                    retranspose(g, qv[g][:])

            # one more matvec, then Rayleigh quotient L = lam * (v'Qv)/(v'v) + eps
            num = [small.tile([GRP, 1], F32, name=f"num{g}") for g in range(N_GROUPS)]
            den = [small.tile([GRP, 1], F32, name=f"den{g}") for g in range(N_GROUPS)]
            for g in range(N_GROUPS):
                matvec(g, with_mu=False)
            for g in range(N_GROUPS):
                nc.vector.scalar_tensor_tensor(
                    trash[g][:], qv[g][:], 0.0, pv[g][:], OP.add, OP.mult,
                    accum_out=num[g][:])
                nc.vector.scalar_tensor_tensor(
                    trash[g][:], qv[g][:], 0.0, qv[g][:], OP.add, OP.mult,
                    accum_out=den[g][:])
                nc.vector.reciprocal(den[g][:], den[g][:])
                # lammax = num/den ; L = lam*lammax + 1e-6 ; nega = -lam/L
                nc.vector.tensor_scalar(num[g][:], num[g][:], den[g][:], None,
                                        OP.mult)
                nc.vector.tensor_scalar(num[g][:], num[g][:], LAMBDA, None,
                                        OP.mult)
                nc.vector.tensor_scalar(num[g][:], num[g][:], 1e-6, None,
                                        OP.add)
                nc.vector.reciprocal(num[g][:], num[g][:])
                nc.vector.tensor_scalar(nega[g][:], num[g][:], -LAMBDA, None,
                                        OP.mult)

            # ---------------- phase C: FISTA ----------------
            for g in range(N_GROUPS):
                nc.vector.memset(yv[g][:], 1.0 / N_ASSETS)
                nc.vector.memset(wprev[g][:], 1.0 / N_ASSETS)
                retranspose(g, yv[g][:])

            wcur, wold = wv, wprev
            for k in range(n_fista):
                ck = cs[k]
                for g in range(N_GROUPS):
                    matvec(g, with_mu=True)
                for g in range(N_GROUPS):
                    if debug and k == 0:
                        dbgt = small.tile([GRP, N_ASSETS], F32, name=f"dbgt{g}",
                                          tag=f"dbgt{g}")
                        nc.vector.tensor_copy(dbgt[:], pv[g][:])
                        nc.sync.dma_start(dbg_d[g * GRP:(g + 1) * GRP, :], dbgt[:])
                    # v = y - a*P   (P = Qy - mu/lam, in PSUM)
                    if k == 0:
                        nc.vector.scalar_tensor_tensor(
                            vv[g][:], pv[g][:], nega[g][:], yv[g][:],
                            OP.mult, OP.add, accum_out=rr[g][:])
                        # cold start: th = (sum(v) - 1)/n
                        nc.vector.tensor_scalar(
                            th[g][:], rr[g][:], -1.0, None, OP.add)
                        nc.vector.tensor_scalar(
                            th[g][:], th[g][:], 1.0 / N_ASSETS, None, OP.mult)
                        n_mich = MICH_COLD
                    else:
                        nc.vector.scalar_tensor_tensor(
                            vv[g][:], pv[g][:], nega[g][:], yv[g][:],
                            OP.mult, OP.add)
                        n_mich = MICH_WARM
                    for _ in range(n_mich):
                        nc.vector.scalar_tensor_tensor(
                            trash[g][:], vv[g][:], th[g][:], zeros8[:],
                            OP.subtract, OP.max, accum_out=rr[g][:])
                        nc.vector.tensor_scalar(
                            trash[g][:], vv[g][:], th[g][:], 0.0,
                            OP.is_gt, OP.add, accum_out=cc[g][:])
                        nc.vector.reciprocal(rc[g][:], cc[g][:])
                        nc.vector.tensor_scalar(
                            dth[g][:], rr[g][:], -1.0, rc[g][:], OP.add, OP.mult)
                        nc.vector.tensor_tensor(
                            th[g][:], th[g][:], dth[g][:], OP.add)
                    # w = relu(v - th)
                    nc.vector.scalar_tensor_tensor(
                        wcur[g][:], vv[g][:], th[g][:], zeros8[:],
                        OP.subtract, OP.max)
                    if k < n_fista - 1:
                        # y = w + ck*(w - wold);  d stored in trash
                        nc.vector.tensor_tensor(
                            trash[g][:], wcur[g][:], wold[g][:], OP.subtract)
                        nc.vector.scalar_tensor_tensor(
                            yv[g][:], trash[g][:], ck, wcur[g][:],
                            OP.mult, OP.add)
                        retranspose(g, yv[g][:])
                wcur, wold = wold, wcur

            # ---------------- output: w / (sum(w) + 1e-12) ----------------
            wfin = wold  # last written group tiles
            for g in range(N_GROUPS):
                wout = small.tile([GRP, N_ASSETS], F32, name=f"wout{g}")
                nc.vector.tensor_scalar(
                    trash[g][:], wfin[g][:], 0.0, 0.0, OP.add, OP.add,
                    accum_out=rr[g][:])
                nc.vector.tensor_scalar(rr[g][:], rr[g][:], 1e-12, None, OP.add)
                nc.vector.reciprocal(rc[g][:], rr[g][:])
                nc.vector.tensor_scalar(
                    wout[:], wfin[g][:], rc[g][:], None, OP.mult)
                nc.sync.dma_start(w_d[g * GRP:(g + 1) * GRP, :], wout[:])
                if debug:
                    dbg2 = small.tile([GRP, 8], F32, name=f"dbg2_{g}")
                    nc.vector.tensor_copy(dbg2[:, 0:1], nega[g][:])
                    nc.vector.tensor_copy(dbg2[:, 1:2], th[g][:])
                    nc.vector.tensor_copy(dbg2[:, 2:3], rr[g][:])
                    nc.vector.tensor_copy(dbg2[:, 3:4], rc[g][:])
                    nc.vector.memset(dbg2[:, 4:8], 0.0)
                    nc.sync.dma_start(dbg2_d[g * GRP:(g + 1) * GRP, :], dbg2[:])

    nc.compile()
    return nc


def get_nc(n_fista=FISTA_ITERS, n_power=POWER_ITERS, debug=False):
    key = (n_fista, n_power, debug)
    if key not in _CACHE:
        _CACHE[key] = _build(n_fista, n_power, debug)
    return _CACHE[key]


def kernel(mu: np.ndarray, U: np.ndarray) -> np.ndarray:
    from concourse.bass_utils import run_bass_kernel_spmd

    nc = get_nc()
    mu = np.ascontiguousarray(mu, dtype=np.float32)
    U = np.ascontiguousarray(U, dtype=np.float32)
    in_maps = [
        {"mu": mu[c * B_CORE:(c + 1) * B_CORE],
         "U": U[c * B_CORE:(c + 1) * B_CORE]}
        for c in range(N_CORES)
    ]
    res = run_bass_kernel_spmd(nc, in_maps, list(range(N_CORES)))
    return np.concatenate([res.results[c]["W"] for c in range(N_CORES)], axis=0)


# revision 5
# speedup vs baseline: 1.3469x; 1.3469x over previous
"""Trainium2 Bass kernel for DifferentiableMVOLayer (batched simplex-constrained QP).

Per-sample FISTA solve of  min -mu'w + (lam/2) w'(U^T U)w  s.t. w in simplex.
Data-parallel over 8 NeuronCores (16 samples each).

Key design points:
  - Q = U^T U formed on-chip (fp32r matmuls, fp32 PSUM accumulate), kept in SBUF.
  - Batched matvec Q@y via masked-stationary trick: stationary [128,8] holds one
    sample's y slice in column b (zeros elsewhere) so 32 matmuls + an identity
    matmul folding -mu/lam accumulate all 8 samples' results into one PSUM tile
    [8, 512] in natural layout.
  - Simplex projection via warm-started Newton/Michelot threshold iteration
    (exact after a few steps), all on the vector engine with fused
    scalar_tensor_tensor/accum ops.
  - FISTA momentum scalars are input-independent -> baked in as immediates.
  - Two 8-sample groups pipeline PE (matvec) against DVE (projection).
"""

import math
import numpy as np

N_ASSETS = 512
BATCH = 128
N_CORES = 8
B_CORE = BATCH // N_CORES          # 16 samples per core
GRP = 8                            # samples per pipeline group
N_GROUPS = B_CORE // GRP
LAMBDA = 10.0
FISTA_ITERS = 150
POWER_ITERS = 16
MICH_COLD = 8                      # Michelot iters, first FISTA step
MICH_WARM = 2                      # Michelot iters, warm-started steps
NT = N_ASSETS // 128               # 4 j-tiles

_CACHE = {}


def _momentum_coeffs(n_iters):
    t = np.float32(1.0)
    cs = []
    for _ in range(n_iters):
        t_new = np.float32(0.5 * (1.0 + np.sqrt(np.float32(1.0 + 4.0 * t * t))))
        cs.append(float((t - np.float32(1.0)) / t_new))
        t = t_new
    return cs


def _build(n_fista, n_power, debug=False):
    import concourse.bass as bass
    import concourse.mybir as mybir
    import concourse.tile as tile
    import concourse.bacc as bacc

    F32 = mybir.dt.float32
    F32R = mybir.dt.float32r
    OP = mybir.AluOpType

    nc = bacc.Bacc(trn_type="TRN2", target_bir_lowering=False)
    mu_d = nc.dram_tensor("mu", [B_CORE, N_ASSETS], F32, kind="ExternalInput")
    u_d = nc.dram_tensor("U", [B_CORE, N_ASSETS, N_ASSETS], F32, kind="ExternalInput")
    w_d = nc.dram_tensor("W", [B_CORE, N_ASSETS], F32, kind="ExternalOutput")
    if debug:
        dbg_d = nc.dram_tensor("DBG", [B_CORE, N_ASSETS], F32, kind="ExternalOutput")
        dbg2_d = nc.dram_tensor("DBG2", [B_CORE, 8], F32, kind="ExternalOutput")

    inv_sqrt_n = 1.0 / math.sqrt(N_ASSETS)
    cs = _momentum_coeffs(n_fista)

    with tile.TileContext(nc) as tc:
        with (
            tc.tile_pool(name="big", bufs=1) as big,
            tc.tile_pool(name="small", bufs=1) as small,
            tc.tile_pool(name="ps", bufs=1, space="PSUM") as ps,
        ):
            # ---------------- static tiles ----------------
            qall = big.tile([128, B_CORE, NT, N_ASSETS], F32R, name="qall")
            mu_sb = small.tile([B_CORE, N_ASSETS], F32, name="mu_sb")
            negmulam = small.tile([B_CORE, N_ASSETS], F32R, name="negmulam")
            zeros8 = small.tile([GRP, N_ASSETS], F32, name="zeros8")
            i16tmp = small.tile([16, 16], F32, name="i16tmp")
            i16f = small.tile([16, 16], F32, name="i16f")
            i16r = small.tile([16, 16], F32R, name="i16r")
            vinit = small.tile([128, NT, GRP], F32, name="vinit")

            nc.sync.dma_start(mu_sb[:], mu_d[:])
            nc.vector.memset(zeros8[:], 0.0)
            nc.gpsimd.iota(i16tmp[:], pattern=[[1, 16]], base=0,
                           channel_multiplier=-1,
                           allow_small_or_imprecise_dtypes=True)
            nc.vector.tensor_scalar(i16f[:], i16tmp[:], 0.0, None, OP.is_equal)
            nc.vector.tensor_copy(i16r[:], i16f[:])
            nc.vector.memset(vinit[:], inv_sqrt_n)
            nc.vector.tensor_scalar(negmulam[:], mu_sb[:], -1.0 / LAMBDA, None,
                                    OP.mult)
            i8 = i16f[0:GRP, 0:GRP]

            # per-group state
            ymask, yv, wv, wprev, vv, trash, yT = [], [], [], [], [], [], []
            th, rr, cc, rc, dth, nega, pv = [], [], [], [], [], [], []
            for g in range(N_GROUPS):
                ymask.append(big.tile([128, NT, GRP, GRP], F32R, name=f"ymask{g}"))
                yv.append(small.tile([GRP, N_ASSETS], F32, name=f"y{g}"))
                wv.append(small.tile([GRP, N_ASSETS], F32, name=f"w{g}"))
                wprev.append(small.tile([GRP, N_ASSETS], F32, name=f"wprev{g}"))
                vv.append(small.tile([GRP, N_ASSETS], F32, name=f"v{g}"))
                trash.append(small.tile([GRP, N_ASSETS], F32, name=f"trash{g}"))
                th.append(small.tile([GRP, 1], F32, name=f"th{g}"))
                rr.append(small.tile([GRP, 1], F32, name=f"r{g}"))
                cc.append(small.tile([GRP, 1], F32, name=f"c{g}"))
                rc.append(small.tile([GRP, 1], F32, name=f"rc{g}"))
                dth.append(small.tile([GRP, 1], F32, name=f"dth{g}"))
                nega.append(small.tile([GRP, 1], F32, name=f"nega{g}"))
                pv.append(ps.tile([GRP, N_ASSETS], F32, name=f"pv{g}"))
                yT.append(ps.tile([128, NT, GRP], F32, name=f"yT{g}"))

            def ym_diag(g):
                return ymask[g][:].rearrange("p t a b -> p t (a b)")[:, :, 0:GRP * GRP:GRP + 1]

            # ---------------- phase A: Q = U^T U ----------------
            with (
                tc.tile_pool(name="stage", bufs=2) as stage_pool,
                tc.tile_pool(name="qps", bufs=4, space="PSUM") as qps_pool,
            ):
                for s in range(B_CORE):
                    ustage = stage_pool.tile([128, NT, N_ASSETS], F32,
                                             name="ustage", tag="ustage")
                    ur = stage_pool.tile([128, NT, N_ASSETS], F32R,
                                         name="ur", tag="ur")
                    nc.sync.dma_start(
                        ustage[:], u_d[s].rearrange("(t p) j -> p t j", p=128))
                    nc.vector.tensor_copy(ur[:], ustage[:])
                    for jm in range(NT):
                        qp = qps_pool.tile([128, N_ASSETS], F32, name="qp", tag="qp")
                        for it in range(NT):
                            nc.tensor.matmul(
                                qp[:], ur[:, it, jm * 128:(jm + 1) * 128],
                                ur[:, it, :],
                                start=(it == 0), stop=(it == NT - 1))
                        nc.vector.tensor_copy(qall[:, s, jm, :], qp[:])

            # ---------------- matvec helper ----------------
            def matvec(g, with_mu):
                for jt in range(NT):
                    for b in range(GRP):
                        s = g * GRP + b
                        last = (jt == NT - 1 and b == GRP - 1 and not with_mu)
                        nc.tensor.matmul(
                            pv[g][:], ymask[g][:, jt, :, b], qall[:, s, jt, :],
                            start=(jt == 0 and b == 0), stop=last)
                if with_mu:
                    nc.tensor.matmul(
                        pv[g][:], i16r[:, g * GRP:(g + 1) * GRP], negmulam[:],
                        start=False, stop=True)

            def retranspose(g, src):
                # src [GRP, 512] fp32 -> ymask diag (fp32r)
                for jt in range(NT):
                    nc.tensor.transpose(
                        yT[g][:, jt, :], src[:, jt * 128:(jt + 1) * 128], i8)
                nc.vector.tensor_copy(ym_diag(g), yT[g][:])

            # ---------------- phase B: power iteration ----------------
            qv = [small.tile([GRP, N_ASSETS], F32, name=f"qv{g}")
                  for g in range(N_GROUPS)]
            ss = [small.tile([GRP, 1], F32, name=f"ss{g}") for g in range(N_GROUPS)]
            sqs = [small.tile([GRP, 1], F32, name=f"sq{g}") for g in range(N_GROUPS)]

            for g in range(N_GROUPS):
                nc.vector.memset(ymask[g][:].bitcast(F32), 0.0)
                nc.vector.tensor_copy(ym_diag(g), vinit[:])

            for it in range(n_power):
                for g in range(N_GROUPS):
                    matvec(g, with_mu=False)
                for g in range(N_GROUPS):
                    nc.vector.tensor_copy(qv[g][:], pv[g][:])
                    nc.vector.scalar_tensor_tensor(
                        trash[g][:], qv[g][:], 0.0, qv[g][:], OP.add, OP.mult,
                        accum_out=ss[g][:])
                    nc.scalar.sqrt(sqs[g][:], ss[g][:])
                    nc.vector.tensor_scalar(sqs[g][:], sqs[g][:], 1e-12, None,
                                            OP.add)
                    nc.vector.reciprocal(rc[g][:], sqs[g][:])
                    nc.vector.tensor_scalar(qv[g][:], qv[g][:], rc[g][:], None,
                                            OP.mult)
                    retranspose(g, qv[g][:])

            # one more matvec, then Rayleigh quotient L = lam * (v'Qv)/(v'v) + eps
            num = [small.tile([GRP, 1], F32, name=f"num{g}") for g in range(N_GROUPS)]
            den = [small.tile([GRP, 1], F32, name=f"den{g}") for g in range(N_GROUPS)]
            for g in range(N_GROUPS):
                matvec(g, with_mu=False)
            for g in range(N_GROUPS):
                nc.vector.scalar_tensor_tensor(
                    trash[g][:], qv[g][:], 0.0, pv[g][:], OP.add, OP.mult,
                    accum_out=num[g][:])
                nc.vector.scalar_tensor_tensor(
                    trash[g][:], qv[g][:], 0.0, qv[g][:], OP.add, OP.mult,
                    accum_out=den[g][:])
                nc.vector.reciprocal(den[g][:], den[g][:])
                # lammax = num/den ; L = lam*lammax + 1e-6 ; nega = -lam/L
                nc.vector.tensor_scalar(num[g][:], num[g][:], den[g][:], None,
                                        OP.mult)
                nc.vector.tensor_scalar(num[g][:], num[g][:], LAMBDA, None,
                                        OP.mult)
                nc.vector.tensor_scalar(num[g][:], num[g][:], 1e-6, None,
                                        OP.add)
                nc.vector.reciprocal(num[g][:], num[g][:])
                nc.vector.tensor_scalar(nega[g][:], num[g][:], -LAMBDA, None,
                                        OP.mult)

            # ---------------- phase C: FISTA ----------------
            for g in range(N_GROUPS):
                nc.vector.memset(yv[g][:], 1.0 / N_ASSETS)
                nc.vector.memset(wprev[g][:], 1.0 / N_ASSETS)
                retranspose(g, yv[g][:])

            wcur, wold = wv, wprev
            for k in range(n_fista):
                ck = cs[k]
                for g in range(N_GROUPS):
                    matvec(g, with_mu=True)
                for g in range(N_GROUPS):
                    if debug and k == 0:
                        dbgt = small.tile([GRP, N_ASSETS], F32, name=f"dbgt{g}",
                                          tag=f"dbgt{g}")
                        nc.vector.tensor_copy(dbgt[:], pv[g][:])
                        nc.sync.dma_start(dbg_d[g * GRP:(g + 1) * GRP, :], dbgt[:])
                    # v = y - a*P   (P = Qy - mu/lam, in PSUM)
                    if k == 0:
                        nc.vector.scalar_tensor_tensor(
                            vv[g][:], pv[g][:], nega[g][:], yv[g][:],
                            OP.mult, OP.add, accum_out=rr[g][:])
                        # cold start: th = (sum(v) - 1)/n
                        nc.vector.tensor_scalar(
                            th[g][:], rr[g][:], -1.0, None, OP.add)
                        nc.vector.tensor_scalar(
                            th[g][:], th[g][:], 1.0 / N_ASSETS, None, OP.mult)
                        n_mich = MICH_COLD
                    else:
                        nc.vector.scalar_tensor_tensor(
                            vv[g][:], pv[g][:], nega[g][:], yv[g][:],
                            OP.mult, OP.add)
                        n_mich = MICH_WARM
                    for _ in range(n_mich):
                        nc.vector.scalar_tensor_tensor(
                            trash[g][:], vv[g][:], th[g][:], zeros8[:],
                            OP.subtract, OP.max, accum_out=rr[g][:])
                        nc.vector.tensor_scalar(
                            trash[g][:], vv[g][:], th[g][:], 0.0,
                            OP.is_gt, OP.add, accum_out=cc[g][:])
                        nc.vector.reciprocal(rc[g][:], cc[g][:])
                        nc.vector.tensor_scalar(
                            dth[g][:], rr[g][:], -1.0, rc[g][:], OP.add, OP.mult)
                        nc.vector.tensor_tensor(
                            th[g][:], th[g][:], dth[g][:], OP.add)
                    # w = relu(v - th)
                    nc.vector.scalar_tensor_tensor(
                        wcur[g][:], vv[g][:], th[g][:], zeros8[:],
                        OP.subtract, OP.max)
                    if k < n_fista - 1:
                        # y = w + ck*(w - wold);  d stored in trash
                        nc.vector.tensor_tensor(
                            trash[g][:], wcur[g][:], wold[g][:], OP.subtract)
                        nc.vector.scalar_tensor_tensor(
                            yv[g][:], trash[g][:], ck, wcur[g][:],
                            OP.mult, OP.add)
                        retranspose(g, yv[g][:])
                wcur, wold = wold, wcur

            # ---------------- output: w / (sum(w) + 1e-12) ----------------
            wfin = wold  # last written group tiles
            for g in range(N_GROUPS):
                wout = small.tile([GRP, N_ASSETS], F32, name=f"wout{g}")
                nc.vector.tensor_scalar(
                    trash[g][:], wfin[g][:], 0.0, 0.0, OP.add, OP.add,
                    accum_out=rr[g][:])
                nc.vector.tensor_scalar(rr[g][:], rr[g][:], 1e-12, None, OP.add)
                nc.vector.reciprocal(rc[g][:], rr[g][:])
                nc.vector.tensor_scalar(
                    wout[:], wfin[g][:], rc[g][:], None, OP.mult)
                nc.sync.dma_start(w_d[g * GRP:(g + 1) * GRP, :], wout[:])
                if debug:
                    dbg2 = small.tile([GRP, 8], F32, name=f"dbg2_{g}")
                    nc.vector.tensor_copy(dbg2[:, 0:1], nega[g][:])
                    nc.vector.tensor_copy(dbg2[:, 1:2], th[g][:])
                    nc.vector.tensor_copy(dbg2[:, 2:3], rr[g][:])
                    nc.vector.tensor_copy(dbg2[:, 3:4], rc[g][:])
                    nc.vector.memset(dbg2[:, 4:8], 0.0)
                    nc.sync.dma_start(dbg2_d[g * GRP:(g + 1) * GRP, :], dbg2[:])

    nc.compile()
    return nc


def get_nc(n_fista=FISTA_ITERS, n_power=POWER_ITERS, debug=False):
    key = (n_fista, n_power, debug)
    if key not in _CACHE:
        _CACHE[key] = _build(n_fista, n_power, debug)
    return _CACHE[key]


def kernel(mu: np.ndarray, U: np.ndarray) -> np.ndarray:
    from concourse.bass_utils import run_bass_kernel_spmd

    nc = get_nc()
    mu = np.ascontiguousarray(mu, dtype=np.float32)
    U = np.ascontiguousarray(U, dtype=np.float32)
    in_maps = [
        {"mu": mu[c * B_CORE:(c + 1) * B_CORE],
         "U": U[c * B_CORE:(c + 1) * B_CORE]}
        for c in range(N_CORES)
    ]
    res = run_bass_kernel_spmd(nc, in_maps, list(range(N_CORES)))
    return np.concatenate([res.results[c]["W"] for c in range(N_CORES)], axis=0)
